# revision 1
# baseline (speedup 1.0000x reference)
"""Trainium2 Bass kernel for nn_MultiHeadAttention (B=4, S=2048, D=1024, H=16).

Sharding: 8 cores = 4 batches x 2 head-groups. Core c handles batch c//2,
heads [8*(c%2), 8*(c%2)+8). Each core computes qkv for its 8 heads,
attention, and a partial c_proj product using its 512 rows of W_proj.
Host sums the two partials per batch (the "all-reduce after c_proj").

Device-side layout choices (per core):
  - x arrives pre-transposed from host: xt = x[b].T  [D=1024, S=2048]
  - Q^T, K^T [128(=2 heads x 64), 4 pairs, S] fp32, V [128(s blk), 16, 512] bf16
  - scores^T per (pair, kb, qchunk) in PSUM, exp on ScalarE -> P^T bf16
  - attn^T accumulated in PSUM via V^T @ P^T (col-group packed head pairs)
  - softmax denominator: DVE bf16 accumulation of P^T over kb + ones-matmul
    partition reduce; no max subtraction (logits ~ N(0, 1/9), tiny)
  - c_proj: attn^T tiles feed matmul lhsT directly, partial out to DRAM
"""

import contextlib
import ctypes
import os
import sys
import types

import numpy as np

# ---------------------------------------------------------------------------
# NTFF profiling hook (used when BASS_PROBLEM_TRACE=1): the agent image lacks
# antenv.axon_hooks, so provide it via ctypes against libaxon_pjrt.so.
# ---------------------------------------------------------------------------
_AXON_SO = "/opt/axon/libaxon_pjrt.so"


def _install_ntff_hook():
    if "antenv.axon_hooks" in sys.modules:
        return
    try:
        import antenv
    except ImportError:
        return
    try:
        lib = ctypes.CDLL(_AXON_SO)
    except OSError:
        return
    if not hasattr(lib, "axon_start_nrt_profile"):
        return
    lib.axon_start_nrt_profile.argtypes = [
        ctypes.POINTER(ctypes.c_int64),
        ctypes.c_size_t,
    ]
    lib.axon_start_nrt_profile.restype = ctypes.c_int64
    lib.axon_stop_nrt_profile.argtypes = [ctypes.c_char_p]
    lib.axon_stop_nrt_profile.restype = ctypes.c_int64

    @contextlib.contextmanager
    def _hook(output_dir, device_ids):
        import jax

        jax.devices()
        if device_ids:
            ids = (ctypes.c_int64 * len(device_ids))(*device_ids)
            rc = lib.axon_start_nrt_profile(ids, len(device_ids))
        else:
            rc = lib.axon_start_nrt_profile(None, 0)
        if rc != 0:
            raise RuntimeError(f"axon_start_nrt_profile rc={rc}")
        try:
            yield
        finally:
            n = lib.axon_stop_nrt_profile(str(output_dir).encode())
            print(f"profile: {n} file(s) written to {output_dir}", file=sys.stderr)

    mod = types.ModuleType("antenv.axon_hooks")
    holder = [_hook]
    mod.get_axon_ntff_profile_hook = lambda: holder[0]
    mod.set_axon_ntff_profile_hook = lambda h: holder.__setitem__(0, h)
    sys.modules["antenv.axon_hooks"] = mod
    antenv.axon_hooks = mod


_install_ntff_hook()

# ---------------------------------------------------------------------------
# Problem constants (hardcoded per the contract)
# ---------------------------------------------------------------------------
B, S, D = 4, 2048, 1024
H, DK = 16, 64
N_CORES = 8
HPC = 8           # heads per core
NPAIR = HPC // 2  # head pairs per core = 4
FC = HPC * DK     # features per core = 512
SCALE = 1.0 / float(np.sqrt(DK))  # 0.125

_CACHED = {}


def _build():
    import concourse.tile as tile
    from concourse import bacc, mybir

    f32 = mybir.dt.float32
    f32r = mybir.dt.float32r
    bf16 = mybir.dt.bfloat16
    Exp = mybir.ActivationFunctionType.Exp

    nc = bacc.Bacc("TRN2", target_bir_lowering=False, debug=False,
                   num_devices=N_CORES)

    xt = nc.dram_tensor("xt", [D, S], f32r, kind="ExternalInput").ap()
    wq = nc.dram_tensor("wq", [D, FC], f32r, kind="ExternalInput").ap()
    wk = nc.dram_tensor("wk", [D, FC], f32r, kind="ExternalInput").ap()
    wv = nc.dram_tensor("wv", [D, FC], f32r, kind="ExternalInput").ap()
    wp = nc.dram_tensor("wp", [128, NPAIR, D], f32r, kind="ExternalInput").ap()
    out = nc.dram_tensor("out", [S, D], f32, kind="ExternalOutput").ap()

    KC = D // 128      # 8 contraction chunks for qkv
    SB = S // 128      # 16 seq blocks
    NQC = 2            # q chunks per seq
    QCW = S // NQC     # q chunk width = 1024
    KB = S // 128      # 16 key blocks

    with tile.TileContext(nc) as tc:
        with (
            tc.tile_pool(name="qkt", bufs=2) as qkt_pool,
            tc.tile_pool(name="vtl", bufs=1) as v_pool,
            tc.tile_pool(name="cst", bufs=1) as cst_pool,
            tc.tile_pool(name="xts", bufs=2) as xts_pool,
            tc.tile_pool(name="wqs", bufs=2) as wqs_pool,
        ):
            Vt = [v_pool.tile([128, 4, FC], bf16, tag=f"V{vc}",
                              name=f"V_{vc}") for vc in range(4)]
            ones = cst_pool.tile([128, 1], bf16, tag="ones")
            nc.gpsimd.memset(ones[:], 1.0)

            def load_xt_sc(sc):
                """Stream one 512-column slice of x^T: [128, KC, 512]."""
                t = xts_pool.tile([128, KC, 512], f32r, tag="XTs",
                                  name=f"xts_{sc}")
                for kc in range(KC):
                    nc.sync.dma_start(
                        t[:, kc, :],
                        xt[kc * 128:(kc + 1) * 128, sc * 512:(sc + 1) * 512])
                return t

            def load_w_pair(src_ap, p, nm):
                """One pair's [128, KC, 128] slice of wq/wk."""
                t = wqs_pool.tile([128, KC, 128], f32r, tag="Wslice",
                                  name=f"w_{nm}_{p}")
                for kc in range(KC):
                    nc.sync.dma_start(
                        t[:, kc, :],
                        src_ap[kc * 128:(kc + 1) * 128, p * 128:(p + 1) * 128])
                return t

            def emit_qk_pair_chunks(p, ps_pool, ps_bufs_tag):
                """Allocate QT/KT tiles for pair p; return (tiles, chunk
                emitters) — each chunk computes one 512-wide seq slice."""
                qtp = qkt_pool.tile([128, S], f32r, tag="QTp", name=f"qtp_{p}")
                ktp = qkt_pool.tile([128, S], f32r, tag="KTp", name=f"ktp_{p}")
                state = {}

                def emit_chunk(sc, p=p, xts=None):
                    if "wq" not in state:
                        state["wq"] = load_w_pair(wq, p, "q")
                        state["wk"] = load_w_pair(wk, p, "k")
                    if xts is None:
                        xts = load_xt_sc(sc)
                    ssl = slice(sc * 512, (sc + 1) * 512)
                    qps = ps_pool.tile([128, 512], f32, tag=ps_bufs_tag,
                                       name=f"qk_ps_{p}_{sc}_q")
                    for kc in range(KC):
                        nc.tensor.matmul(
                            qps[:], lhsT=state["wq"][:, kc, :],
                            rhs=xts[:, kc, :],
                            start=(kc == 0), stop=(kc == KC - 1))
                    nc.vector.tensor_copy(qtp[:, ssl], qps[:])
                    kps = ps_pool.tile([128, 512], f32, tag=ps_bufs_tag,
                                       name=f"qk_ps_{p}_{sc}_k")
                    for kc in range(KC):
                        nc.tensor.matmul(
                            kps[:], lhsT=state["wk"][:, kc, :],
                            rhs=xts[:, kc, :],
                            start=(kc == 0), stop=(kc == KC - 1))
                    nc.vector.tensor_copy(ktp[:, ssl], kps[:])

                return (qtp, ktp), emit_chunk

            # ---------------- prologue: V + pair-0 Q/K -------------------
            qk = {}
            with (
                tc.tile_pool(name="wvp", bufs=1) as wv_pool,
                tc.tile_pool(name="ps1", bufs=2, space="PSUM") as ps1,
            ):
                WV = wv_pool.tile([128, KC, FC], f32r, tag="WV")
                for kc in range(KC):
                    nc.sync.dma_start(WV[:, kc, :],
                                      wv[kc * 128:(kc + 1) * 128, :])
                qk[0], qk0_chunk = emit_qk_pair_chunks(0, ps1, "vps")
                for sc in range(4):
                    xts = load_xt_sc(sc)
                    qk0_chunk(sc, xts=xts)
                    for sbl in range(4):
                        vps = ps1.tile([128, FC], f32, tag="vps",
                                       name=f"vps_{sc}_{sbl}")
                        for kc in range(KC):
                            nc.tensor.matmul(
                                vps[:],
                                lhsT=xts[:, kc, sbl * 128:(sbl + 1) * 128],
                                rhs=WV[:, kc, :],
                                start=(kc == 0), stop=(kc == KC - 1))
                        nc.vector.tensor_copy(Vt[sc][:, sbl, :], vps[:])

            # ---------------- attention + interleaved qkv ----------------
            with (
                tc.tile_pool(name="atn", bufs=1) as attn_pool,
                tc.tile_pool(name="wpp", bufs=1) as wp_pool,
            ):
                ATN = attn_pool.tile([128, NPAIR, S], f32r, tag="ATN")
                WP = wp_pool.tile([128, NPAIR, D], f32r, tag="WP")
                for p in range(NPAIR):
                    nc.sync.dma_start(WP[:, p, :], wp[:, p, :])

                with (
                    tc.tile_pool(name="ptp", bufs=10) as pt_pool,
                    tc.tile_pool(name="dnp", bufs=2) as den_pool,
                    tc.tile_pool(name="dnq", bufs=1) as den2_pool,
                    tc.tile_pool(name="ivp", bufs=1) as inv_pool,
                    tc.tile_pool(name="stp", bufs=2, space="PSUM") as st_ps,
                    tc.tile_pool(name="avp", bufs=2, space="PSUM") as av_pool,
                    tc.tile_pool(name="dsp", bufs=1, space="PSUM") as dps_pool,
                    tc.tile_pool(name="qps2", bufs=1, space="PSUM") as qkv_ps,
                ):
                    def emit_boundary(pend):
                        """Close out a finished (p, qc) block: denominator
                        reduce, reciprocal, broadcast, divide -> ATN."""
                        (bp, bqc, bavs, baccA, baccB) = pend
                        bqoff = bqc * QCW
                        den = den2_pool.tile([128, 2, QCW], bf16, tag="den",
                                             name=f"den_{bp}_{bqc}")
                        nc.vector.tensor_add(den[:], baccA[:], baccB[:])
                        inv = inv_pool.tile([1, 2, QCW], f32, tag="inv",
                                            name=f"inv_{bp}_{bqc}")
                        for h in range(2):
                            for ns in range(2):
                                dps = dps_pool.tile([1, 512], f32, tag="dps",
                                                    name=f"dps_{bp}_{bqc}_{h}_{ns}")
                                nc.tensor.matmul(
                                    dps[:], lhsT=ones[:],
                                    rhs=den[:, h, ns * 512:(ns + 1) * 512],
                                    start=True, stop=True)
                                nc.vector.reciprocal_approx_fast(
                                    inv[0:1, h, ns * 512:(ns + 1) * 512],
                                    dps[:])
                        ibc = inv_pool.tile([128, 2, QCW], f32, tag="ibc",
                                            name=f"ibc_{bp}_{bqc}")
                        for h in range(2):
                            nc.gpsimd.partition_broadcast(
                                ibc[:, h, :], inv[0:1, h, :])
                        for h in range(2):
                            hsl = slice(64 * h, 64 * h + 64)
                            for ns in range(2):
                                nc.vector.tensor_mul(
                                    ATN[hsl, bp,
                                        bqoff + ns * 512:bqoff + (ns + 1) * 512],
                                    bavs[ns][hsl, :],
                                    ibc[hsl, h, ns * 512:(ns + 1) * 512])

                    pending = None
                    for p in range(NPAIR):
                        QTp, KTp = qk.pop(p)
                        for qc in range(NQC):
                            qoff = qc * QCW
                            avs = [av_pool.tile([128, 512], f32, tag="avps",
                                                name=f"avps_{p}_{qc}_{ns}")
                                   for ns in range(2)]
                            accA = den_pool.tile([128, 2, QCW], bf16,
                                                 tag="accA",
                                                 name=f"accA_{p}_{qc}")
                            accB = den_pool.tile([128, 2, QCW], bf16,
                                                 tag="accB",
                                                 name=f"accB_{p}_{qc}")
                            pts = {}

                            def sc_exp_den(kb, p=p, qc=qc, qoff=qoff,
                                           QTp=QTp, KTp=KTp,
                                           accA=accA, accB=accB, pts=pts):
                                for ns in range(2):
                                    nsl = slice(qoff + ns * 512,
                                                qoff + (ns + 1) * 512)
                                    st = st_ps.tile([128, 2, 512], f32,
                                                    tag="st",
                                                    name=f"st_{p}_{qc}_{kb}_{ns}")
                                    for h in range(2):
                                        hsl = slice(64 * h, 64 * h + 64)
                                        nc.tensor.matmul(
                                            st[:, h, :],
                                            lhsT=KTp[hsl,
                                                     kb * 128:(kb + 1) * 128],
                                            rhs=QTp[hsl, nsl],
                                            start=True, stop=True)
                                    pt = pt_pool.tile(
                                        [128, 2, 512], bf16, tag="pt",
                                        name=f"pt_{p}_{qc}_{kb}_{ns}")
                                    nc.scalar.activation(pt[:], st[:], Exp,
                                                         scale=SCALE)
                                    pts[(kb, ns)] = pt
                                    # denominator partials: A chain on DVE,
                                    # B chain on the otherwise-idle GpSimd
                                    acc = accA if kb % 2 == 0 else accB
                                    asl = acc[:, :, ns * 512:(ns + 1) * 512]
                                    if kb < 2:
                                        nc.vector.tensor_copy(asl, pt[:])
                                    else:
                                        nc.vector.tensor_add(asl, asl, pt[:])

                            def av(kb, p=p, avs=avs, pts=pts):
                                for ns in range(2):
                                    pt = pts.pop((kb, ns))
                                    for h in range(2):
                                        nc.tensor.matmul(
                                            avs[ns][64 * h:64 * h + 64, :],
                                            lhsT=Vt[kb // 4][
                                                :, kb % 4,
                                                p * 128 + 64 * h:
                                                p * 128 + 64 * h + 64],
                                            rhs=pt[:, h, :],
                                            start=(kb == 0),
                                            stop=(kb == KB - 1),
                                            tile_position=(0, 64 * h),
                                            skip_group_check=True)

                            # head start on scores/exp before closing out the
                            # previous block, so ScalarE never starves
                            filler = None
                            if qc == 0 and p + 1 < NPAIR:
                                qk[p + 1], filler = emit_qk_pair_chunks(
                                    p + 1, qkv_ps, "qk2")
                            sc_exp_den(0)
                            sc_exp_den(1)
                            sc_exp_den(2)
                            if pending is not None:
                                emit_boundary(pending)
                            av(0)
                            av(1)
                            av(2)
                            for kb in range(3, KB):
                                sc_exp_den(kb)
                                av(kb)
                                if filler is not None and kb in (3, 6, 9, 12):
                                    filler((kb - 3) // 3)
                            pending = (p, qc, avs, accA, accB)
                    emit_boundary(pending)

                # ---------------- c_proj partial -------------------------
                with (
                    tc.tile_pool(name="osb", bufs=3) as out_pool,
                    tc.tile_pool(name="pjp", bufs=2, space="PSUM") as pj_ps,
                ):
                    for sb in range(SB):
                        for nn in range(2):
                            pps = pj_ps.tile([128, 512], f32, tag="pps")
                            for p in range(NPAIR):
                                nc.tensor.matmul(
                                    pps[:],
                                    lhsT=ATN[:, p, sb * 128:(sb + 1) * 128],
                                    rhs=WP[:, p, nn * 512:(nn + 1) * 512],
                                    start=(p == 0), stop=(p == NPAIR - 1))
                            ot = out_pool.tile([128, 512], f32, tag="ot")
                            nc.vector.tensor_copy(ot[:], pps[:])
                            nc.sync.dma_start(
                                out[sb * 128:(sb + 1) * 128,
                                    nn * 512:(nn + 1) * 512],
                                ot[:])

    nc.compile()
    return nc


def _get_nc():
    if "nc" not in _CACHED:
        _CACHED["nc"] = _build()
    return _CACHED["nc"]


def _to_f32r(a):
    """Round fp32 to the fp32r grid (1s + 8e + 11m, low 12 bits zero, RNE)."""
    u = np.ascontiguousarray(a, dtype=np.float32).view(np.uint32).astype(np.uint64)
    u = (u + 0x7FF + ((u >> 12) & 1)) & 0xFFFFF000
    return u.astype(np.uint32).view(np.float32)


def _shard(x, W_attn, W_proj):
    """Build per-core input maps."""
    x = np.asarray(x, dtype=np.float32)
    W_attn = np.asarray(W_attn, dtype=np.float32)
    W_proj = np.asarray(W_proj, dtype=np.float32)
    in_maps = []
    for c in range(N_CORES):
        b, g = c // 2, c % 2
        fsl = slice(g * FC, (g + 1) * FC)
        in_maps.append({
            "xt": _to_f32r(x[b].T),
            "wq": _to_f32r(W_attn[:, 0 * D + g * FC:0 * D + (g + 1) * FC]),
            "wk": _to_f32r(W_attn[:, 1 * D + g * FC:1 * D + (g + 1) * FC]),
            "wv": _to_f32r(W_attn[:, 2 * D + g * FC:2 * D + (g + 1) * FC]),
            "wp": _to_f32r(
                W_proj[fsl, :].reshape(NPAIR, 128, D).transpose(1, 0, 2)),
        })
    return in_maps


def kernel(x, W_attn, W_proj):
    from concourse.bass_utils import run_bass_kernel_spmd

    nc = _get_nc()
    in_maps = _shard(x, W_attn, W_proj)
    trace = os.environ.get("BASS_PROBLEM_TRACE", "0") == "1"
    res = run_bass_kernel_spmd(nc, in_maps, list(range(N_CORES)), trace=trace)
    _CACHED["last_result"] = res
    out = np.empty((B, S, D), dtype=np.float32)
    for b in range(B):
        out[b] = res.results[2 * b]["out"] + res.results[2 * b + 1]["out"]
    return out



# revision 5
# speedup vs baseline: 1.1084x; 1.1084x over previous
"""Trainium2 Bass kernel for nn_MultiHeadAttention (B=4, S=2048, D=1024, H=16).

Sharding: 8 cores = 4 batches x 2 head-groups. Core c handles batch c//2,
heads [8*(c%2), 8*(c%2)+8). Each core computes qkv for its 8 heads,
attention, and a partial c_proj using its 512 rows of W_proj. Host sums
the two partials per batch (the "all-reduce after c_proj").

v2 layout (all-bf16 matmul path, f32 PSUM accumulation):
  - whole x^T resident in SBUF [128, 8, 2048] bf16; weights resident too
  - per pair p: Q^T, K^T [128(=2 heads x 64), S] bf16
  - V^T [128(key blk), 16, 512] bf16, built just-in-time during the first
    attention block
  - attention in (p, qc) blocks of 512 q columns: per key block kb one
    scores PSUM tile [128, 2, 512] (h-major), one exp -> pt bf16; pt tiles
    pair two kb so denominator adds run as single [128, 2048] DVE instrs
  - denominator: bf16 A/B accumulation chains + ones-matmul partition
    reduce (no max subtraction; logits are tiny)
  - filler chains (V JIT, next-pair QK, per-qc c_proj) drain from a queue
    at 1 per 2 kb iterations to keep the PE busy through the scalar-bound
    attention blocks
"""

import contextlib
import ctypes
import os
import sys
import types
from collections import deque

import numpy as np

# ---------------------------------------------------------------------------
# NTFF profiling hook (used when BASS_PROBLEM_TRACE=1): the agent image lacks
# antenv.axon_hooks, so provide it via ctypes against libaxon_pjrt.so.
# ---------------------------------------------------------------------------
_AXON_SO = "/opt/axon/libaxon_pjrt.so"


def _install_ntff_hook():
    if "antenv.axon_hooks" in sys.modules:
        return
    try:
        import antenv
    except ImportError:
        return
    try:
        lib = ctypes.CDLL(_AXON_SO)
    except OSError:
        return
    if not hasattr(lib, "axon_start_nrt_profile"):
        return
    lib.axon_start_nrt_profile.argtypes = [
        ctypes.POINTER(ctypes.c_int64),
        ctypes.c_size_t,
    ]
    lib.axon_start_nrt_profile.restype = ctypes.c_int64
    lib.axon_stop_nrt_profile.argtypes = [ctypes.c_char_p]
    lib.axon_stop_nrt_profile.restype = ctypes.c_int64

    @contextlib.contextmanager
    def _hook(output_dir, device_ids):
        import jax

        jax.devices()
        if device_ids:
            ids = (ctypes.c_int64 * len(device_ids))(*device_ids)
            rc = lib.axon_start_nrt_profile(ids, len(device_ids))
        else:
            rc = lib.axon_start_nrt_profile(None, 0)
        if rc != 0:
            raise RuntimeError(f"axon_start_nrt_profile rc={rc}")
        try:
            yield
        finally:
            n = lib.axon_stop_nrt_profile(str(output_dir).encode())
            print(f"profile: {n} file(s) written to {output_dir}", file=sys.stderr)

    mod = types.ModuleType("antenv.axon_hooks")
    holder = [_hook]
    mod.get_axon_ntff_profile_hook = lambda: holder[0]
    mod.set_axon_ntff_profile_hook = lambda h: holder.__setitem__(0, h)
    sys.modules["antenv.axon_hooks"] = mod
    antenv.axon_hooks = mod


_install_ntff_hook()

# ---------------------------------------------------------------------------
# Problem constants (hardcoded per the contract)
# ---------------------------------------------------------------------------
B, S, D = 4, 2048, 1024
H, DK = 16, 64
N_CORES = 8
HPC = 8           # heads per core
NPAIR = HPC // 2  # head pairs per core = 4
FC = HPC * DK     # features per core = 512
SCALE = 1.0 / float(np.sqrt(DK))  # 0.125

KC = D // 128     # 8 contraction chunks
KB = S // 128     # 16 key blocks
NQC = 4           # q chunks per pair
QCW = S // NQC    # 512

_CACHED = {}


def _build():
    import concourse.tile as tile
    from concourse import bacc, mybir

    f32 = mybir.dt.float32
    bf16 = mybir.dt.bfloat16
    Exp = mybir.ActivationFunctionType.Exp

    nc = bacc.Bacc("TRN2", target_bir_lowering=False, debug=False,
                   num_devices=N_CORES)

    xt = nc.dram_tensor("xt", [D, S], bf16, kind="ExternalInput").ap()
    wq = nc.dram_tensor("wq", [D, FC], bf16, kind="ExternalInput").ap()
    wk = nc.dram_tensor("wk", [D, FC], bf16, kind="ExternalInput").ap()
    wv = nc.dram_tensor("wv", [D, FC], bf16, kind="ExternalInput").ap()
    wp = nc.dram_tensor("wp", [128, NPAIR, D], bf16, kind="ExternalInput").ap()
    out = nc.dram_tensor("out", [S, D], f32, kind="ExternalOutput").ap()

    with tile.TileContext(nc) as tc:
        with (
            tc.tile_pool(name="res", bufs=1) as res_pool,
            tc.tile_pool(name="qkt", bufs=2) as qkt_pool,
            tc.tile_pool(name="ptp", bufs=4) as pt_pool,
            tc.tile_pool(name="acc", bufs=2) as acc_pool,
            tc.tile_pool(name="dnq", bufs=2) as den_pool,
            tc.tile_pool(name="ivp", bufs=2) as inv_pool,
            tc.tile_pool(name="ibp", bufs=2) as ibc_pool,
            tc.tile_pool(name="osb", bufs=3) as out_pool,
            tc.tile_pool(name="stp", bufs=2, space="PSUM") as st_ps,
            tc.tile_pool(name="avp", bufs=2, space="PSUM") as av_ps,
            tc.tile_pool(name="chn", bufs=1, space="PSUM") as chain_ps,
            tc.tile_pool(name="dsp", bufs=1, space="PSUM") as dps_ps,
        ):
            # ------------- resident SBUF tensors + input DMA -------------
            XT = res_pool.tile([128, KC, S], bf16, tag="XT")
            WQ = res_pool.tile([128, KC, FC], bf16, tag="WQ")
            WK = res_pool.tile([128, KC, FC], bf16, tag="WK")
            WV = res_pool.tile([128, KC, FC], bf16, tag="WV")
            WP = res_pool.tile([128, NPAIR, D], bf16, tag="WP")
            VT = res_pool.tile([128, KB, FC], bf16, tag="VT")
            ATN = res_pool.tile([128, NPAIR, S], bf16, tag="ATN")
            ones = res_pool.tile([128, 1], bf16, tag="ones")
            nc.gpsimd.memset(ones[:], 1.0)

            # first-needed first: x first half + WQ/WK, then the rest
            for kc in range(KC):
                nc.sync.dma_start(XT[:, kc, 0:1024],
                                  xt[kc * 128:(kc + 1) * 128, 0:1024])
            for kc in range(KC):
                nc.sync.dma_start(WQ[:, kc, :], wq[kc * 128:(kc + 1) * 128, :])
                nc.sync.dma_start(WK[:, kc, :], wk[kc * 128:(kc + 1) * 128, :])
            for kc in range(KC):
                nc.sync.dma_start(XT[:, kc, 1024:2048],
                                  xt[kc * 128:(kc + 1) * 128, 1024:2048])
                nc.sync.dma_start(WV[:, kc, :], wv[kc * 128:(kc + 1) * 128, :])
            for p in range(NPAIR):
                nc.sync.dma_start(WP[:, p, :], wp[:, p, :])

            QT = {}
            KT = {}

            # ------------- chain emitters (tensor matmul chains) ----------
            def emit_qk_chain(dst, w, p, sc, nm):
                """One 512-col slice of Q^T or K^T for pair p."""
                ps = chain_ps.tile([128, 512], f32, tag="chain",
                                   name=f"qk_{nm}_{p}_{sc}")
                ssl = slice(sc * 512, (sc + 1) * 512)
                for kc in range(KC):
                    nc.tensor.matmul(ps[:], lhsT=w[:, kc, p * 128:(p + 1) * 128],
                                     rhs=XT[:, kc, ssl],
                                     start=(kc == 0), stop=(kc == KC - 1))
                nc.vector.tensor_copy(dst[:, ssl], ps[:])

            def build_qk(p):
                """Allocate QT/KT tiles for pair p; return chain closures."""
                QT[p] = qkt_pool.tile([128, S], bf16, tag="QT", name=f"QT{p}")
                KT[p] = qkt_pool.tile([128, S], bf16, tag="KT", name=f"KT{p}")
                return [
                    (lambda sc=sc, w=w, d=d, nm=nm: emit_qk_chain(d, w, p, sc, nm))
                    for (w, d, nm) in ((WQ, QT[p], "q"), (WK, KT[p], "k"))
                    for sc in range(4)
                ]

            def emit_v_chain(kb):
                """V^T for key block kb: [128 keys, FC]."""
                ps = chain_ps.tile([128, FC], f32, tag="chain", name=f"v_{kb}")
                for kc in range(KC):
                    nc.tensor.matmul(ps[:],
                                     lhsT=XT[:, kc, kb * 128:(kb + 1) * 128],
                                     rhs=WV[:, kc, :],
                                     start=(kc == 0), stop=(kc == KC - 1))
                nc.vector.tensor_copy(VT[:, kb, :], ps[:])

            def emit_cproj_chain(sb, nn):
                """c_proj for 128 q rows x 512 out cols."""
                ps = chain_ps.tile([128, 512], f32, tag="chain",
                                   name=f"cp_{sb}_{nn}")
                for p in range(NPAIR):
                    nc.tensor.matmul(ps[:],
                                     lhsT=ATN[:, p, sb * 128:(sb + 1) * 128],
                                     rhs=WP[:, p, nn * 512:(nn + 1) * 512],
                                     start=(p == 0), stop=(p == NPAIR - 1))
                ot = out_pool.tile([128, 512], f32, tag="ot",
                                   name=f"ot_{sb}_{nn}")
                nc.vector.tensor_copy(ot[:], ps[:])
                nc.sync.dma_start(
                    out[sb * 128:(sb + 1) * 128, nn * 512:(nn + 1) * 512],
                    ot[:])

            def cproj_fillers(qc):
                return [(lambda sb=sb, nn=nn: emit_cproj_chain(sb, nn))
                        for sb in range(qc * 4, qc * 4 + 4) for nn in range(2)]

            # ------------- boundary: softmax denominator + normalize ------
            def emit_boundary(pend):
                (bp, bqc, bavs, baccA, baccB) = pend
                qsl = slice(bqc * QCW, (bqc + 1) * QCW)
                den = den_pool.tile([128, 2, 2, 512], bf16, tag="den",
                                    name=f"den_{bp}_{bqc}")
                nc.vector.tensor_add(den[:], baccA[:], baccB[:])
                inv = inv_pool.tile([1, 2, 512], f32, tag="inv",
                                    name=f"inv_{bp}_{bqc}")
                for h in range(2):
                    dps = dps_ps.tile([1, 512], f32, tag="dps",
                                      name=f"dps_{bp}_{bqc}_{h}")
                    for sl in range(2):
                        nc.tensor.matmul(dps[:], lhsT=ones[:],
                                         rhs=den[:, h, sl, :],
                                         start=(sl == 0), stop=(sl == 1))
                    nc.vector.reciprocal_approx_fast(inv[0:1, h, :], dps[:])
                ib = ibc_pool.tile([128, 2, 512], f32, tag="ibc",
                                   name=f"ibc_{bp}_{bqc}")
                for h in range(2):
                    nc.gpsimd.partition_broadcast(ib[:, h, :], inv[0:1, h, :])
                for h in range(2):
                    hsl = slice(64 * h, 64 * h + 64)
                    nc.vector.tensor_mul(ATN[hsl, bp, qsl], bavs[hsl, :],
                                         ib[hsl, h, :])

            # ------------- prologue: pair-0 Q/K + first V blocks ----------
            for f in build_qk(0):
                f()
            for kb in range(4):
                emit_v_chain(kb)

            filler_q = deque()
            pending = None
            for p in range(NPAIR):
                for qc in range(NQC):
                    if p < NPAIR - 1 and qc == 1:
                        filler_q.extend(build_qk(p + 1))
                    if p == NPAIR - 1 and qc >= 1:
                        filler_q.extend(cproj_fillers(qc - 1))

                    avs = av_ps.tile([128, QCW], f32, tag="avps",
                                     name=f"avps_{p}_{qc}")
                    accA = acc_pool.tile([128, 2, 2, 512], bf16, tag="accA",
                                         name=f"accA_{p}_{qc}")
                    accB = acc_pool.tile([128, 2, 2, 512], bf16, tag="accB",
                                         name=f"accB_{p}_{qc}")
                    pts = {}

                    def sc_exp(kb, p=p, qc=qc, pts=pts, accA=accA, accB=accB):
                        slot = kb % 2
                        st = st_ps.tile([128, 2, 512], f32, tag="st",
                                        name=f"st_{p}_{qc}_{kb}")
                        for h in range(2):
                            hsl = slice(64 * h, 64 * h + 64)
                            nc.tensor.matmul(
                                st[:, h, :],
                                lhsT=KT[p][hsl, kb * 128:(kb + 1) * 128],
                                rhs=QT[p][hsl, qc * QCW:(qc + 1) * QCW],
                                start=True, stop=True)
                        if slot == 0:
                            pt = pt_pool.tile([128, 2, 2, 512], bf16, tag="pt",
                                              name=f"pt_{p}_{qc}_{kb}")
                            pts[kb // 2] = pt
                        else:
                            pt = pts[kb // 2]
                        nc.scalar.activation(pt[:, :, slot, :], st[:], Exp,
                                             scale=SCALE)
                        if slot == 1:
                            pr = kb // 2
                            acc = accA if pr % 2 == 0 else accB
                            if pr < 2:
                                nc.vector.tensor_copy(acc[:], pt[:])
                            else:
                                nc.vector.tensor_add(acc[:], acc[:], pt[:])

                    def av(kb, p=p, avs=avs, pts=pts):
                        pt = pts[kb // 2]
                        for h in range(2):
                            nc.tensor.matmul(
                                avs[64 * h:64 * h + 64, :],
                                lhsT=VT[:, kb,
                                        p * 128 + 64 * h:p * 128 + 64 * h + 64],
                                rhs=pt[:, h, kb % 2, :],
                                start=(kb == 0), stop=(kb == KB - 1),
                                tile_position=(0, 64 * h),
                                skip_group_check=True)

                    sc_exp(0)
                    sc_exp(1)
                    if pending is not None:
                        emit_boundary(pending)
                    av(0)
                    av(1)
                    for kb in range(2, KB):
                        sc_exp(kb)
                        av(kb)
                        if p == 0 and qc == 0:
                            if kb < 14:
                                emit_v_chain(kb + 2)
                        elif kb % 2 == 0 and filler_q:
                            filler_q.popleft()()
                    pending = (p, qc, avs, accA, accB)

            emit_boundary(pending)
            while filler_q:
                filler_q.popleft()()
            for f in cproj_fillers(NQC - 1):
                f()

    nc.compile()
    return nc


def _get_nc():
    if "nc" not in _CACHED:
        _CACHED["nc"] = _build()
    return _CACHED["nc"]


def _shard(x, W_attn, W_proj):
    """Build per-core input maps (bf16)."""
    import ml_dtypes

    bf = ml_dtypes.bfloat16
    x = np.asarray(x, dtype=np.float32)
    W_attn = np.asarray(W_attn, dtype=np.float32)
    W_proj = np.asarray(W_proj, dtype=np.float32)
    in_maps = []
    for c in range(N_CORES):
        b, g = c // 2, c % 2
        fsl = slice(g * FC, (g + 1) * FC)
        in_maps.append({
            "xt": np.ascontiguousarray(x[b].T).astype(bf),
            "wq": np.ascontiguousarray(
                W_attn[:, 0 * D + g * FC:0 * D + (g + 1) * FC]).astype(bf),
            "wk": np.ascontiguousarray(
                W_attn[:, 1 * D + g * FC:1 * D + (g + 1) * FC]).astype(bf),
            "wv": np.ascontiguousarray(
                W_attn[:, 2 * D + g * FC:2 * D + (g + 1) * FC]).astype(bf),
            "wp": np.ascontiguousarray(
                W_proj[fsl, :].reshape(NPAIR, 128, D).transpose(1, 0, 2)
            ).astype(bf),
        })
    return in_maps


def kernel(x, W_attn, W_proj):
    from concourse.bass_utils import run_bass_kernel_spmd

    nc = _get_nc()
    in_maps = _shard(x, W_attn, W_proj)
    trace = os.environ.get("BASS_PROBLEM_TRACE", "0") == "1"
    res = run_bass_kernel_spmd(nc, in_maps, list(range(N_CORES)), trace=trace)
    _CACHED["last_result"] = res
    out = np.empty((B, S, D), dtype=np.float32)
    for b in range(B):
        out[b] = res.results[2 * b]["out"] + res.results[2 * b + 1]["out"]
    return out


# revision 11
# speedup vs baseline: 1.1662x; 1.0521x over previous
"""Trainium2 Bass kernel for nn_MultiHeadAttention (B=4, S=2048, D=1024, H=16).

Sharding: 8 cores = 4 batches x 2 head-groups. Core c handles batch c//2,
heads [8*(c%2), 8*(c%2)+8). Each core computes qkv for its 8 heads,
attention, and a partial c_proj using its 512 rows of W_proj. Host sums
the two partials per batch (the "all-reduce after c_proj").

v3 (all-bf16 matmul path, f32 PSUM accumulation):
  - whole x^T + all weights resident in SBUF; per-kc tiles so the first
    qkv chains start as soon as their own DMA slices land
  - per pair p: Q^T, K^T [128(=2 heads x 64), S] bf16, built via filler
    chains just-in-time; V^T likewise during the first attention block
  - attention in (p, qc) blocks of 512 q columns; per key block kb one
    scores PSUM tile [128, 2, 512] (h-major), one exp -> pt bf16; pt pairs
    two kb so denominator adds are single [128, 2048] DVE instrs
  - softmax denominator: bf16 A/B chains, folded on DVE, partition-reduced
    on GpSimd (partition_all_reduce) -- no tensor-engine ones-matmuls
  - c_proj per qc chunk as filler once all pairs finish that chunk;
    bf16 partial output, host sums in f32
"""

import contextlib
import ctypes
import os
import sys
import types
from collections import deque

import numpy as np

# ---------------------------------------------------------------------------
# NTFF profiling hook (used when BASS_PROBLEM_TRACE=1): the agent image lacks
# antenv.axon_hooks, so provide it via ctypes against libaxon_pjrt.so.
# ---------------------------------------------------------------------------
_AXON_SO = "/opt/axon/libaxon_pjrt.so"


def _install_ntff_hook():
    if "antenv.axon_hooks" in sys.modules:
        return
    try:
        import antenv
    except ImportError:
        return
    try:
        lib = ctypes.CDLL(_AXON_SO)
    except OSError:
        return
    if not hasattr(lib, "axon_start_nrt_profile"):
        return
    lib.axon_start_nrt_profile.argtypes = [
        ctypes.POINTER(ctypes.c_int64),
        ctypes.c_size_t,
    ]
    lib.axon_start_nrt_profile.restype = ctypes.c_int64
    lib.axon_stop_nrt_profile.argtypes = [ctypes.c_char_p]
    lib.axon_stop_nrt_profile.restype = ctypes.c_int64

    @contextlib.contextmanager
    def _hook(output_dir, device_ids):
        import jax

        jax.devices()
        if device_ids:
            ids = (ctypes.c_int64 * len(device_ids))(*device_ids)
            rc = lib.axon_start_nrt_profile(ids, len(device_ids))
        else:
            rc = lib.axon_start_nrt_profile(None, 0)
        if rc != 0:
            raise RuntimeError(f"axon_start_nrt_profile rc={rc}")
        try:
            yield
        finally:
            n = lib.axon_stop_nrt_profile(str(output_dir).encode())
            print(f"profile: {n} file(s) written to {output_dir}", file=sys.stderr)

    mod = types.ModuleType("antenv.axon_hooks")
    holder = [_hook]
    mod.get_axon_ntff_profile_hook = lambda: holder[0]
    mod.set_axon_ntff_profile_hook = lambda h: holder.__setitem__(0, h)
    sys.modules["antenv.axon_hooks"] = mod
    antenv.axon_hooks = mod


_install_ntff_hook()

# ---------------------------------------------------------------------------
# Problem constants (hardcoded per the contract)
# ---------------------------------------------------------------------------
B, S, D = 4, 2048, 1024
H, DK = 16, 64
N_CORES = 8
HPC = 8           # heads per core
NPAIR = HPC // 2  # head pairs per core = 4
FC = HPC * DK     # features per core = 512
SCALE = 1.0 / float(np.sqrt(DK))  # 0.125

KC = D // 128     # 8 contraction chunks
KB = S // 128     # 16 key blocks
NQC = 4           # q chunks per pair
QCW = S // NQC    # 512

_CACHED = {}


def _build():
    import concourse.tile as tile
    from concourse import bacc, bass_isa, mybir

    f32 = mybir.dt.float32
    bf16 = mybir.dt.bfloat16
    Exp = mybir.ActivationFunctionType.Exp
    RAdd = bass_isa.ReduceOp.add

    nc = bacc.Bacc("TRN2", target_bir_lowering=False, debug=False,
                   num_devices=N_CORES)

    xt = nc.dram_tensor("xt", [D, S], bf16, kind="ExternalInput").ap()
    wq = nc.dram_tensor("wq", [D, FC], bf16, kind="ExternalInput").ap()
    wk = nc.dram_tensor("wk", [D, FC], bf16, kind="ExternalInput").ap()
    wv = nc.dram_tensor("wv", [D, FC], bf16, kind="ExternalInput").ap()
    wp = nc.dram_tensor("wp", [128, NPAIR, D], bf16, kind="ExternalInput").ap()
    out = nc.dram_tensor("out", [S, D], bf16, kind="ExternalOutput").ap()

    with tile.TileContext(nc) as tc:
        with (
            tc.tile_pool(name="res", bufs=1) as res_pool,
            tc.tile_pool(name="qkt", bufs=3) as qkt_pool,
            tc.tile_pool(name="ptp", bufs=4) as pt_pool,
            tc.tile_pool(name="acc", bufs=2) as acc_pool,
            tc.tile_pool(name="dnq", bufs=2) as den_pool,
            tc.tile_pool(name="ibp", bufs=2) as ibc_pool,
            tc.tile_pool(name="rcp", bufs=2) as rec_pool,
            tc.tile_pool(name="osb", bufs=3) as out_pool,
            tc.tile_pool(name="stp", bufs=2, space="PSUM") as st_ps,
            tc.tile_pool(name="avp", bufs=2, space="PSUM") as av_ps,
            tc.tile_pool(name="chn", bufs=1, space="PSUM") as chain_ps,
            tc.tile_pool(name="dsp", bufs=1, space="PSUM") as dps_ps,
        ):
            # ------------- resident SBUF tensors ------------------------
            XTk = [res_pool.tile([128, S], bf16, tag=f"XT{kc}",
                                 name=f"XT{kc}") for kc in range(KC)]
            WQk = [res_pool.tile([128, FC], bf16, tag=f"WQ{kc}",
                                 name=f"WQ{kc}") for kc in range(KC)]
            WKk = [res_pool.tile([128, FC], bf16, tag=f"WK{kc}",
                                 name=f"WK{kc}") for kc in range(KC)]
            WVk = [res_pool.tile([128, FC], bf16, tag=f"WV{kc}",
                                 name=f"WV{kc}") for kc in range(KC)]
            WP = res_pool.tile([128, NPAIR, D], bf16, tag="WP")
            VT = res_pool.tile([128, KB, FC], bf16, tag="VT")
            ATN = res_pool.tile([128, NPAIR, S], bf16, tag="ATN")
            ones = res_pool.tile([128, 1], bf16, tag="ones")
            nc.gpsimd.memset(ones[:], 1.0)

            # input DMA, first-needed first: x sc0 + WQ per kc, then WK
            # (first K chain), then the rest
            for kc in range(KC):
                nc.sync.dma_start(XTk[kc][:, 0:512],
                                  xt[kc * 128:(kc + 1) * 128, 0:512])
                nc.sync.dma_start(WQk[kc][:], wq[kc * 128:(kc + 1) * 128, :])
            for kc in range(KC):
                nc.sync.dma_start(WKk[kc][:], wk[kc * 128:(kc + 1) * 128, :])
                nc.sync.dma_start(WVk[kc][:], wv[kc * 128:(kc + 1) * 128, :])
            for kc in range(KC):
                nc.sync.dma_start(XTk[kc][:, 512:1024],
                                  xt[kc * 128:(kc + 1) * 128, 512:1024])
            for kc in range(KC):
                nc.sync.dma_start(XTk[kc][:, 1024:1536],
                                  xt[kc * 128:(kc + 1) * 128, 1024:1536])
                nc.sync.dma_start(XTk[kc][:, 1536:2048],
                                  xt[kc * 128:(kc + 1) * 128, 1536:2048])
            for p in range(NPAIR):
                nc.sync.dma_start(WP[:, p, :], wp[:, p, :])

            QT = {}
            KT = {}

            # ------------- chain emitters (tensor matmul chains) ----------
            def emit_qk_chain(dst, wk_tiles, p, sc, nm):
                """One 512-col slice of Q^T or K^T for pair p."""
                ps = chain_ps.tile([128, 512], f32, tag="chain",
                                   name=f"qk_{nm}_{p}_{sc}")
                ssl = slice(sc * 512, (sc + 1) * 512)
                for kc in range(KC):
                    nc.tensor.matmul(ps[:],
                                     lhsT=wk_tiles[kc][:, p * 128:(p + 1) * 128],
                                     rhs=XTk[kc][:, ssl],
                                     start=(kc == 0), stop=(kc == KC - 1))
                nc.vector.tensor_copy(dst[:, ssl], ps[:])

            def build_qk(p):
                QT[p] = qkt_pool.tile([128, S], bf16, tag="QT", name=f"QT{p}")
                KT[p] = qkt_pool.tile([128, S], bf16, tag="KT", name=f"KT{p}")

            def q_chain(p, sc):
                return lambda: emit_qk_chain(QT[p], WQk, p, sc, "q")

            def k_chain(p, sc):
                return lambda: emit_qk_chain(KT[p], WKk, p, sc, "k")

            def emit_v_chain(kb):
                """V^T for key block kb: [128 keys, FC]."""
                ps = chain_ps.tile([128, FC], f32, tag="chain", name=f"v_{kb}")
                for kc in range(KC):
                    nc.tensor.matmul(ps[:],
                                     lhsT=XTk[kc][:, kb * 128:(kb + 1) * 128],
                                     rhs=WVk[kc][:],
                                     start=(kc == 0), stop=(kc == KC - 1))
                nc.vector.tensor_copy(VT[:, kb, :], ps[:])

            def emit_cproj_chain(sb, nn):
                """c_proj for 128 q rows x 512 out cols."""
                ps = chain_ps.tile([128, 512], f32, tag="chain",
                                   name=f"cp_{sb}_{nn}")
                for p in range(NPAIR):
                    nc.tensor.matmul(ps[:],
                                     lhsT=ATN[:, p, sb * 128:(sb + 1) * 128],
                                     rhs=WP[:, p, nn * 512:(nn + 1) * 512],
                                     start=(p == 0), stop=(p == NPAIR - 1))
                ot = out_pool.tile([128, 512], bf16, tag="ot",
                                   name=f"ot_{sb}_{nn}")
                nc.vector.tensor_copy(ot[:], ps[:])
                nc.sync.dma_start(
                    out[sb * 128:(sb + 1) * 128, nn * 512:(nn + 1) * 512],
                    ot[:])

            def cproj_fillers(qc):
                return [(lambda sb=sb, nn=nn: emit_cproj_chain(sb, nn))
                        for sb in range(qc * 4, qc * 4 + 4) for nn in range(2)]

            # ------------- boundary: softmax denominator + normalize ------
            def emit_boundary(pend):
                (bp, bqc, bavs, baccA, baccB) = pend
                qsl = slice(bqc * QCW, (bqc + 1) * QCW)
                den = den_pool.tile([128, 2, 2, 512], bf16, tag="den",
                                    name=f"den_{bp}_{bqc}")
                nc.vector.tensor_add(den[:], baccA[:], baccB[:])
                inv = rec_pool.tile([1, 2, 512], f32, tag="inv",
                                    name=f"inv_{bp}_{bqc}")
                for h in range(2):
                    dps = dps_ps.tile([1, 512], f32, tag="dps",
                                      name=f"dps_{bp}_{bqc}_{h}")
                    for sl in range(2):
                        nc.tensor.matmul(dps[:], lhsT=ones[:],
                                         rhs=den[:, h, sl, :],
                                         start=(sl == 0), stop=(sl == 1))
                    nc.vector.reciprocal_approx_fast(inv[0:1, h, :], dps[:])
                ib = ibc_pool.tile([128, 2, 512], f32, tag="ibc",
                                   name=f"ibc_{bp}_{bqc}")
                for h in range(2):
                    nc.gpsimd.partition_broadcast(ib[:, h, :], inv[0:1, h, :])
                for h in range(2):
                    hsl = slice(64 * h, 64 * h + 64)
                    nc.vector.tensor_mul(ATN[hsl, bp, qsl], bavs[hsl, :],
                                         ib[hsl, h, :])

            # ------------- prologue -------------------------------------
            build_qk(0)
            q_chain(0, 0)()
            k_chain(0, 0)()
            for kb in range(4):
                emit_v_chain(kb)

            # just-in-time filler list for block (0, 0): (kb key, fn);
            # every V chain lands >=2 iterations ahead of its av(kb) reader
            jit00 = deque([
                (2, lambda: emit_v_chain(4)), (2, lambda: emit_v_chain(5)),
                (2, k_chain(0, 1)),
                (4, lambda: emit_v_chain(6)), (4, lambda: emit_v_chain(7)),
                (5, k_chain(0, 2)), (5, lambda: emit_v_chain(8)),
                (6, lambda: emit_v_chain(9)), (6, lambda: emit_v_chain(10)),
                (7, k_chain(0, 3)), (7, lambda: emit_v_chain(11)),
                (8, lambda: emit_v_chain(12)), (8, lambda: emit_v_chain(13)),
                (9, lambda: emit_v_chain(14)), (9, lambda: emit_v_chain(15)),
                (11, q_chain(0, 1)),
            ])

            filler_q = deque()
            pending = None
            for p in range(NPAIR):
                for qc in range(NQC):
                    # hazard guard: QK chains for pair p must be fully
                    # emitted before this pair's first scores read them
                    if qc == 0 and p > 0:
                        while filler_q:
                            filler_q.popleft()()
                    if p == 0 and qc == 1:
                        filler_q.append(q_chain(0, 2))
                    elif p == 0 and qc == 2:
                        build_qk(1)
                        filler_q.append(q_chain(0, 3))
                        for sc in range(4):
                            filler_q.append(k_chain(1, sc))
                        filler_q.append(q_chain(1, 0))
                    elif p == 0 and qc == 3:
                        for sc in range(1, 4):
                            filler_q.append(q_chain(1, sc))
                    elif p == 1 and qc == 2:
                        build_qk(2)
                        for sc in range(4):
                            filler_q.append(k_chain(2, sc))
                        filler_q.append(q_chain(2, 0))
                    elif p == 1 and qc == 3:
                        for sc in range(1, 4):
                            filler_q.append(q_chain(2, sc))
                    elif p == 2 and qc == 2:
                        build_qk(3)
                        for sc in range(4):
                            filler_q.append(k_chain(3, sc))
                        filler_q.append(q_chain(3, 0))
                    elif p == 2 and qc == 3:
                        for sc in range(1, 4):
                            filler_q.append(q_chain(3, sc))
                    elif p == 3 and qc >= 1:
                        filler_q.extend(cproj_fillers(qc - 1))

                    avs = av_ps.tile([128, QCW], f32, tag="avps",
                                     name=f"avps_{p}_{qc}")
                    accA = acc_pool.tile([128, 2, 2, 512], bf16, tag="accA",
                                         name=f"accA_{p}_{qc}")
                    accB = acc_pool.tile([128, 2, 2, 512], bf16, tag="accB",
                                         name=f"accB_{p}_{qc}")
                    pts = {}

                    def sc_exp(kb, p=p, qc=qc, pts=pts, accA=accA, accB=accB):
                        slot = kb % 2
                        st = st_ps.tile([128, 2, 512], f32, tag="st",
                                        name=f"st_{p}_{qc}_{kb}")
                        for h in range(2):
                            hsl = slice(64 * h, 64 * h + 64)
                            nc.tensor.matmul(
                                st[:, h, :],
                                lhsT=KT[p][hsl, kb * 128:(kb + 1) * 128],
                                rhs=QT[p][hsl, qc * QCW:(qc + 1) * QCW],
                                start=True, stop=True)
                        if slot == 0:
                            pt = pt_pool.tile([128, 2, 2, 512], bf16, tag="pt",
                                              name=f"pt_{p}_{qc}_{kb}")
                            pts[kb // 2] = pt
                        else:
                            pt = pts[kb // 2]
                        nc.scalar.activation(pt[:, :, slot, :], st[:], Exp,
                                             scale=SCALE)
                        if slot == 1:
                            pr = kb // 2
                            acc = accA if pr % 2 == 0 else accB
                            if pr < 2:
                                nc.vector.tensor_copy(acc[:], pt[:])
                            else:
                                nc.vector.tensor_add(acc[:], acc[:], pt[:])

                    def av(kb, p=p, avs=avs, pts=pts):
                        pt = pts[kb // 2]
                        for h in range(2):
                            nc.tensor.matmul(
                                avs[64 * h:64 * h + 64, :],
                                lhsT=VT[:, kb,
                                        p * 128 + 64 * h:p * 128 + 64 * h + 64],
                                rhs=pt[:, h, kb % 2, :],
                                start=(kb == 0), stop=(kb == KB - 1),
                                tile_position=(0, 64 * h),
                                skip_group_check=True)

                    sc_exp(0)
                    sc_exp(1)
                    if pending is not None:
                        emit_boundary(pending)
                    av(0)
                    av(1)
                    for kb in range(2, KB):
                        sc_exp(kb)
                        if p == 0 and qc == 0:
                            while jit00 and jit00[0][0] <= kb:
                                jit00.popleft()[1]()
                        av(kb)
                        if (p, qc) != (0, 0) and kb % 2 == 0 and filler_q:
                            filler_q.popleft()()
                    pending = (p, qc, avs, accA, accB)

            emit_boundary(pending)
            while filler_q:
                filler_q.popleft()()
            for f in cproj_fillers(NQC - 1):
                f()

    nc.compile()
    return nc


def _get_nc():
    if "nc" not in _CACHED:
        _CACHED["nc"] = _build()
    return _CACHED["nc"]


def _shard(x, W_attn, W_proj):
    """Build per-core input maps (bf16)."""
    import ml_dtypes

    bf = ml_dtypes.bfloat16
    x = np.asarray(x, dtype=np.float32)
    W_attn = np.asarray(W_attn, dtype=np.float32)
    W_proj = np.asarray(W_proj, dtype=np.float32)
    in_maps = []
    for c in range(N_CORES):
        b, g = c // 2, c % 2
        fsl = slice(g * FC, (g + 1) * FC)
        in_maps.append({
            "xt": np.ascontiguousarray(x[b].T).astype(bf),
            "wq": np.ascontiguousarray(
                W_attn[:, 0 * D + g * FC:0 * D + (g + 1) * FC]).astype(bf),
            "wk": np.ascontiguousarray(
                W_attn[:, 1 * D + g * FC:1 * D + (g + 1) * FC]).astype(bf),
            "wv": np.ascontiguousarray(
                W_attn[:, 2 * D + g * FC:2 * D + (g + 1) * FC]).astype(bf),
            "wp": np.ascontiguousarray(
                W_proj[fsl, :].reshape(NPAIR, 128, D).transpose(1, 0, 2)
            ).astype(bf),
        })
    return in_maps


def kernel(x, W_attn, W_proj):
    from concourse.bass_utils import run_bass_kernel_spmd

    nc = _get_nc()
    in_maps = _shard(x, W_attn, W_proj)
    trace = os.environ.get("BASS_PROBLEM_TRACE", "0") == "1"
    res = run_bass_kernel_spmd(nc, in_maps, list(range(N_CORES)), trace=trace)
    _CACHED["last_result"] = res
    out = np.empty((B, S, D), dtype=np.float32)
    for b in range(B):
        out[b] = (res.results[2 * b]["out"].astype(np.float32)
                  + res.results[2 * b + 1]["out"].astype(np.float32))
    return out


# revision 14
# speedup vs baseline: 1.2035x; 1.0320x over previous
"""Trainium2 Bass kernel for nn_MultiHeadAttention (B=4, S=2048, D=1024, H=16).

Sharding: 8 cores = 4 batches x 2 head-groups. Core c handles batch c//2,
heads [8*(c%2), 8*(c%2)+8). Each core computes qkv for its 8 heads,
attention, and a partial c_proj using its 512 rows of W_proj. Host sums
the two partials per batch (the "all-reduce after c_proj").

v3 (all-bf16 matmul path, f32 PSUM accumulation):
  - whole x^T + all weights resident in SBUF; per-kc tiles so the first
    qkv chains start as soon as their own DMA slices land
  - per pair p: Q^T, K^T [128(=2 heads x 64), S] bf16, built via filler
    chains just-in-time; V^T likewise during the first attention block
  - attention in (p, qc) blocks of 512 q columns; per key block kb one
    scores PSUM tile [128, 2, 512] (h-major), one exp -> pt bf16; pt pairs
    two kb so denominator adds are single [128, 2048] DVE instrs
  - softmax denominator: bf16 A/B chains, folded on DVE, partition-reduced
    on GpSimd (partition_all_reduce) -- no tensor-engine ones-matmuls
  - c_proj per qc chunk as filler once all pairs finish that chunk;
    bf16 partial output, host sums in f32
"""

import contextlib
import ctypes
import os
import sys
import types
from collections import deque

import numpy as np

# ---------------------------------------------------------------------------
# NTFF profiling hook (used when BASS_PROBLEM_TRACE=1): the agent image lacks
# antenv.axon_hooks, so provide it via ctypes against libaxon_pjrt.so.
# ---------------------------------------------------------------------------
_AXON_SO = "/opt/axon/libaxon_pjrt.so"


def _install_ntff_hook():
    if "antenv.axon_hooks" in sys.modules:
        return
    try:
        import antenv
    except ImportError:
        return
    try:
        lib = ctypes.CDLL(_AXON_SO)
    except OSError:
        return
    if not hasattr(lib, "axon_start_nrt_profile"):
        return
    lib.axon_start_nrt_profile.argtypes = [
        ctypes.POINTER(ctypes.c_int64),
        ctypes.c_size_t,
    ]
    lib.axon_start_nrt_profile.restype = ctypes.c_int64
    lib.axon_stop_nrt_profile.argtypes = [ctypes.c_char_p]
    lib.axon_stop_nrt_profile.restype = ctypes.c_int64

    @contextlib.contextmanager
    def _hook(output_dir, device_ids):
        import jax

        jax.devices()
        if device_ids:
            ids = (ctypes.c_int64 * len(device_ids))(*device_ids)
            rc = lib.axon_start_nrt_profile(ids, len(device_ids))
        else:
            rc = lib.axon_start_nrt_profile(None, 0)
        if rc != 0:
            raise RuntimeError(f"axon_start_nrt_profile rc={rc}")
        try:
            yield
        finally:
            n = lib.axon_stop_nrt_profile(str(output_dir).encode())
            print(f"profile: {n} file(s) written to {output_dir}", file=sys.stderr)

    mod = types.ModuleType("antenv.axon_hooks")
    holder = [_hook]
    mod.get_axon_ntff_profile_hook = lambda: holder[0]
    mod.set_axon_ntff_profile_hook = lambda h: holder.__setitem__(0, h)
    sys.modules["antenv.axon_hooks"] = mod
    antenv.axon_hooks = mod


_install_ntff_hook()

# ---------------------------------------------------------------------------
# Problem constants (hardcoded per the contract)
# ---------------------------------------------------------------------------
B, S, D = 4, 2048, 1024
H, DK = 16, 64
N_CORES = 8
HPC = 8           # heads per core
NPAIR = HPC // 2  # head pairs per core = 4
FC = HPC * DK     # features per core = 512
SCALE = 1.0 / float(np.sqrt(DK))  # 0.125

KC = D // 128     # 8 contraction chunks
KB = S // 128     # 16 key blocks
NQC = 4           # q chunks per pair
QCW = S // NQC    # 512

_CACHED = {}


def _build():
    import concourse.tile as tile
    from concourse import bacc, bass_isa, mybir

    f32 = mybir.dt.float32
    bf16 = mybir.dt.bfloat16
    Exp = mybir.ActivationFunctionType.Exp
    RAdd = bass_isa.ReduceOp.add

    nc = bacc.Bacc("TRN2", target_bir_lowering=False, debug=False,
                   num_devices=N_CORES)

    xt = nc.dram_tensor("xt", [D, S], bf16, kind="ExternalInput").ap()
    wq = nc.dram_tensor("wq", [D, FC], bf16, kind="ExternalInput").ap()
    wk = nc.dram_tensor("wk", [D, FC], bf16, kind="ExternalInput").ap()
    wv = nc.dram_tensor("wv", [D, FC], bf16, kind="ExternalInput").ap()
    wp = nc.dram_tensor("wp", [128, NPAIR, D], bf16, kind="ExternalInput").ap()
    out = nc.dram_tensor("out", [S, D], bf16, kind="ExternalOutput").ap()

    with tile.TileContext(nc) as tc:
        with (
            tc.tile_pool(name="res", bufs=1) as res_pool,
            tc.tile_pool(name="qkt", bufs=3) as qkt_pool,
            tc.tile_pool(name="ptp", bufs=4) as pt_pool,
            tc.tile_pool(name="acc", bufs=2) as acc_pool,
            tc.tile_pool(name="dnq", bufs=2) as den_pool,
            tc.tile_pool(name="ibp", bufs=2) as ibc_pool,
            tc.tile_pool(name="rcp", bufs=2) as rec_pool,
            tc.tile_pool(name="osb", bufs=3) as out_pool,
            tc.tile_pool(name="stp", bufs=2, space="PSUM") as st_ps,
            tc.tile_pool(name="avp", bufs=2, space="PSUM") as av_ps,
            tc.tile_pool(name="chn", bufs=1, space="PSUM") as chain_ps,
            tc.tile_pool(name="dsp", bufs=1, space="PSUM") as dps_ps,
        ):
            # ------------- resident SBUF tensors ------------------------
            XTk = [res_pool.tile([128, S], bf16, tag=f"XT{kc}",
                                 name=f"XT{kc}") for kc in range(KC)]
            WQk = [res_pool.tile([128, FC], bf16, tag=f"WQ{kc}",
                                 name=f"WQ{kc}") for kc in range(KC)]
            WKk = [res_pool.tile([128, FC], bf16, tag=f"WK{kc}",
                                 name=f"WK{kc}") for kc in range(KC)]
            WVk = [res_pool.tile([128, FC], bf16, tag=f"WV{kc}",
                                 name=f"WV{kc}") for kc in range(KC)]
            WP = res_pool.tile([128, NPAIR, D], bf16, tag="WP")
            VT = res_pool.tile([128, KB, FC], bf16, tag="VT")
            ATN = res_pool.tile([128, NPAIR, S], bf16, tag="ATN")
            ones = res_pool.tile([128, 1], bf16, tag="ones")
            nc.gpsimd.memset(ones[:], 1.0)

            # input DMA, first-needed first: everything the first scores
            # matmuls need (x sc0 + WQ + WK, per kc), then WV (first V
            # chains), then the remaining x columns, WP last
            for kc in range(KC):
                nc.sync.dma_start(XTk[kc][:, 0:512],
                                  xt[kc * 128:(kc + 1) * 128, 0:512])
                nc.sync.dma_start(WQk[kc][:], wq[kc * 128:(kc + 1) * 128, :])
                nc.sync.dma_start(WKk[kc][:], wk[kc * 128:(kc + 1) * 128, :])
            for kc in range(KC):
                nc.sync.dma_start(WVk[kc][:], wv[kc * 128:(kc + 1) * 128, :])
            for kc in range(KC):
                nc.sync.dma_start(XTk[kc][:, 512:1024],
                                  xt[kc * 128:(kc + 1) * 128, 512:1024])
            for kc in range(KC):
                nc.sync.dma_start(XTk[kc][:, 1024:1536],
                                  xt[kc * 128:(kc + 1) * 128, 1024:1536])
                nc.sync.dma_start(XTk[kc][:, 1536:2048],
                                  xt[kc * 128:(kc + 1) * 128, 1536:2048])
            for p in range(NPAIR):
                nc.sync.dma_start(WP[:, p, :], wp[:, p, :])

            QT = {}
            KT = {}

            # ------------- chain emitters (tensor matmul chains) ----------
            def emit_qk_chain(dst, wk_tiles, p, sc, nm):
                """One 512-col slice of Q^T or K^T for pair p."""
                ps = chain_ps.tile([128, 512], f32, tag="chain",
                                   name=f"qk_{nm}_{p}_{sc}")
                ssl = slice(sc * 512, (sc + 1) * 512)
                for kc in range(KC):
                    nc.tensor.matmul(ps[:],
                                     lhsT=wk_tiles[kc][:, p * 128:(p + 1) * 128],
                                     rhs=XTk[kc][:, ssl],
                                     start=(kc == 0), stop=(kc == KC - 1))
                nc.vector.tensor_copy(dst[:, ssl], ps[:])

            def build_qk(p):
                QT[p] = qkt_pool.tile([128, S], bf16, tag="QT", name=f"QT{p}")
                KT[p] = qkt_pool.tile([128, S], bf16, tag="KT", name=f"KT{p}")

            def q_chain(p, sc):
                return lambda: emit_qk_chain(QT[p], WQk, p, sc, "q")

            def k_chain(p, sc):
                return lambda: emit_qk_chain(KT[p], WKk, p, sc, "k")

            def emit_v_chain(kb):
                """V^T for key block kb: [128 keys, FC]."""
                ps = chain_ps.tile([128, FC], f32, tag="chain", name=f"v_{kb}")
                for kc in range(KC):
                    nc.tensor.matmul(ps[:],
                                     lhsT=XTk[kc][:, kb * 128:(kb + 1) * 128],
                                     rhs=WVk[kc][:],
                                     start=(kc == 0), stop=(kc == KC - 1))
                nc.vector.tensor_copy(VT[:, kb, :], ps[:])

            def emit_cproj_chain(sb, nn, pool=None):
                """c_proj for 128 q rows x 512 out cols."""
                ps = (pool or chain_ps).tile([128, 512], f32,
                                             tag="chain" if pool is None
                                             else "avps",
                                             name=f"cp_{sb}_{nn}")
                for p in range(NPAIR):
                    nc.tensor.matmul(ps[:],
                                     lhsT=ATN[:, p, sb * 128:(sb + 1) * 128],
                                     rhs=WP[:, p, nn * 512:(nn + 1) * 512],
                                     start=(p == 0), stop=(p == NPAIR - 1))
                ot = out_pool.tile([128, 512], bf16, tag="ot",
                                   name=f"ot_{sb}_{nn}")
                nc.vector.tensor_copy(ot[:], ps[:])
                nc.sync.dma_start(
                    out[sb * 128:(sb + 1) * 128, nn * 512:(nn + 1) * 512],
                    ot[:])

            def cproj_fillers(qc):
                return [(lambda sb=sb, nn=nn: emit_cproj_chain(sb, nn))
                        for sb in range(qc * 4, qc * 4 + 4) for nn in range(2)]

            # ------------- boundary: softmax denominator + normalize ------
            def emit_boundary(pend):
                (bp, bqc, bavs, baccA, baccB) = pend
                qsl = slice(bqc * QCW, (bqc + 1) * QCW)
                den = den_pool.tile([128, 2, 2, 512], bf16, tag="den",
                                    name=f"den_{bp}_{bqc}")
                nc.vector.tensor_add(den[:], baccA[:], baccB[:])
                inv = rec_pool.tile([1, 2, 512], f32, tag="inv",
                                    name=f"inv_{bp}_{bqc}")
                for h in range(2):
                    dps = dps_ps.tile([1, 512], f32, tag="dps",
                                      name=f"dps_{bp}_{bqc}_{h}")
                    for sl in range(2):
                        nc.tensor.matmul(dps[:], lhsT=ones[:],
                                         rhs=den[:, h, sl, :],
                                         start=(sl == 0), stop=(sl == 1))
                    nc.vector.reciprocal_approx_fast(inv[0:1, h, :], dps[:])
                ib = ibc_pool.tile([128, 2, 512], f32, tag="ibc",
                                   name=f"ibc_{bp}_{bqc}")
                for h in range(2):
                    nc.gpsimd.partition_broadcast(ib[:, h, :], inv[0:1, h, :])
                for h in range(2):
                    hsl = slice(64 * h, 64 * h + 64)
                    nc.vector.tensor_mul(ATN[hsl, bp, qsl], bavs[hsl, :],
                                         ib[hsl, h, :])

            # ------------- prologue -------------------------------------
            build_qk(0)
            q_chain(0, 0)()
            k_chain(0, 0)()
            for kb in range(4):
                emit_v_chain(kb)

            # just-in-time filler list for block (0, 0): (kb key, fn);
            # every V chain lands >=2 iterations ahead of its av(kb) reader
            jit00 = deque([
                (2, lambda: emit_v_chain(4)), (2, lambda: emit_v_chain(5)),
                (2, k_chain(0, 1)),
                (4, lambda: emit_v_chain(6)), (4, lambda: emit_v_chain(7)),
                (5, k_chain(0, 2)), (5, lambda: emit_v_chain(8)),
                (6, lambda: emit_v_chain(9)), (6, lambda: emit_v_chain(10)),
                (7, k_chain(0, 3)), (7, lambda: emit_v_chain(11)),
                (8, lambda: emit_v_chain(12)), (8, lambda: emit_v_chain(13)),
                (9, lambda: emit_v_chain(14)), (9, lambda: emit_v_chain(15)),
                (11, q_chain(0, 1)),
            ])

            filler_q = deque()
            pending = None
            for p in range(NPAIR):
                for qc in range(NQC):
                    # hazard guard: QK chains for pair p must be fully
                    # emitted before this pair's first scores read them
                    if qc == 0 and p > 0:
                        while filler_q:
                            filler_q.popleft()()
                    if p == 0 and qc == 1:
                        filler_q.append(q_chain(0, 2))
                    elif p == 0 and qc == 2:
                        build_qk(1)
                        filler_q.append(q_chain(0, 3))
                        for sc in range(4):
                            filler_q.append(k_chain(1, sc))
                        filler_q.append(q_chain(1, 0))
                    elif p == 0 and qc == 3:
                        for sc in range(1, 4):
                            filler_q.append(q_chain(1, sc))
                    elif p == 1 and qc == 2:
                        build_qk(2)
                        for sc in range(4):
                            filler_q.append(k_chain(2, sc))
                        filler_q.append(q_chain(2, 0))
                    elif p == 1 and qc == 3:
                        for sc in range(1, 4):
                            filler_q.append(q_chain(2, sc))
                    elif p == 2 and qc == 2:
                        build_qk(3)
                        for sc in range(4):
                            filler_q.append(k_chain(3, sc))
                        filler_q.append(q_chain(3, 0))
                    elif p == 2 and qc == 3:
                        for sc in range(1, 4):
                            filler_q.append(q_chain(3, sc))
                    elif p == 3 and qc >= 1:
                        filler_q.extend(cproj_fillers(qc - 1))

                    avs = av_ps.tile([128, QCW], f32, tag="avps",
                                     name=f"avps_{p}_{qc}")
                    accA = acc_pool.tile([128, 2, 2, 512], bf16, tag="accA",
                                         name=f"accA_{p}_{qc}")
                    accB = acc_pool.tile([128, 2, 2, 512], bf16, tag="accB",
                                         name=f"accB_{p}_{qc}")
                    pts = {}

                    def sc_exp(kb, p=p, qc=qc, pts=pts, accA=accA, accB=accB):
                        slot = kb % 2
                        st = st_ps.tile([128, 2, 512], f32, tag="st",
                                        name=f"st_{p}_{qc}_{kb}")
                        for h in range(2):
                            hsl = slice(64 * h, 64 * h + 64)
                            nc.tensor.matmul(
                                st[:, h, :],
                                lhsT=KT[p][hsl, kb * 128:(kb + 1) * 128],
                                rhs=QT[p][hsl, qc * QCW:(qc + 1) * QCW],
                                start=True, stop=True)
                        if slot == 0:
                            pt = pt_pool.tile([128, 2, 2, 512], bf16, tag="pt",
                                              name=f"pt_{p}_{qc}_{kb}")
                            pts[kb // 2] = pt
                        else:
                            pt = pts[kb // 2]
                        nc.scalar.activation(pt[:, :, slot, :], st[:], Exp,
                                             scale=SCALE)
                        if slot == 1:
                            pr = kb // 2
                            acc = accA if pr % 2 == 0 else accB
                            if pr < 2:
                                nc.vector.tensor_copy(acc[:], pt[:])
                            else:
                                nc.vector.tensor_add(acc[:], acc[:], pt[:])

                    def av(kb, p=p, avs=avs, pts=pts):
                        pt = pts[kb // 2]
                        for h in range(2):
                            nc.tensor.matmul(
                                avs[64 * h:64 * h + 64, :],
                                lhsT=VT[:, kb,
                                        p * 128 + 64 * h:p * 128 + 64 * h + 64],
                                rhs=pt[:, h, kb % 2, :],
                                start=(kb == 0), stop=(kb == KB - 1),
                                tile_position=(0, 64 * h),
                                skip_group_check=True)

                    sc_exp(0)
                    sc_exp(1)
                    if pending is not None:
                        emit_boundary(pending)
                    av(0)
                    av(1)
                    for kb in range(2, KB):
                        sc_exp(kb)
                        if p == 0 and qc == 0:
                            while jit00 and jit00[0][0] <= kb:
                                jit00.popleft()[1]()
                        av(kb)
                        if (p, qc) != (0, 0) and kb % 2 == 0 and filler_q:
                            filler_q.popleft()()
                    pending = (p, qc, avs, accA, accB)

            emit_boundary(pending)
            while filler_q:
                filler_q.popleft()()
            # tail: alternate PSUM pools so chain->copy->chain never
            # serializes on a single buffer
            qc = NQC - 1
            for j, (sb, nn) in enumerate(
                    [(sb, nn) for sb in range(qc * 4, qc * 4 + 4)
                     for nn in range(2)]):
                emit_cproj_chain(sb, nn, pool=av_ps if j % 2 else None)

    nc.compile()
    return nc


def _get_nc():
    if "nc" not in _CACHED:
        _CACHED["nc"] = _build()
    return _CACHED["nc"]


def _shard(x, W_attn, W_proj):
    """Build per-core input maps (bf16)."""
    import ml_dtypes

    bf = ml_dtypes.bfloat16
    x = np.asarray(x, dtype=np.float32)
    W_attn = np.asarray(W_attn, dtype=np.float32)
    W_proj = np.asarray(W_proj, dtype=np.float32)
    in_maps = []
    for c in range(N_CORES):
        b, g = c // 2, c % 2
        fsl = slice(g * FC, (g + 1) * FC)
        in_maps.append({
            "xt": np.ascontiguousarray(x[b].T).astype(bf),
            "wq": np.ascontiguousarray(
                W_attn[:, 0 * D + g * FC:0 * D + (g + 1) * FC]).astype(bf),
            "wk": np.ascontiguousarray(
                W_attn[:, 1 * D + g * FC:1 * D + (g + 1) * FC]).astype(bf),
            "wv": np.ascontiguousarray(
                W_attn[:, 2 * D + g * FC:2 * D + (g + 1) * FC]).astype(bf),
            "wp": np.ascontiguousarray(
                W_proj[fsl, :].reshape(NPAIR, 128, D).transpose(1, 0, 2)
            ).astype(bf),
        })
    return in_maps


def kernel(x, W_attn, W_proj):
    from concourse.bass_utils import run_bass_kernel_spmd

    nc = _get_nc()
    in_maps = _shard(x, W_attn, W_proj)
    trace = os.environ.get("BASS_PROBLEM_TRACE", "0") == "1"
    res = run_bass_kernel_spmd(nc, in_maps, list(range(N_CORES)), trace=trace)
    _CACHED["last_result"] = res
    out = np.empty((B, S, D), dtype=np.float32)
    for b in range(B):
        out[b] = (res.results[2 * b]["out"].astype(np.float32)
                  + res.results[2 * b + 1]["out"].astype(np.float32))
    return out
